# revision 22
# baseline (speedup 1.0000x reference)
"""Paged-attention decode (GQA) on 8 Trainium2 NeuronCores.

Sharding: tensor-parallel along the kv-head axis. Core i gets kv head i
and its 4 query heads (H=32, KVH=8 -> G=4), plus all 64 sequences.

Host-side prep (per core) — a per-shard block re-allocator:
  - scatter the new k/v token into the cache shard (store_kvcache)
  - defragment: order each sequence's allocated blocks contiguously,
    dropping blocks past ceil(context_len/128) (never attended)
  - K laid out [d, seq-chunk-major slots] so K^T streams into SBUF
    with d on partitions (the QK^T matmul contracts over d)
  - V laid out [slot-in-chunk, seq-chunk-major (d+1)] with a ones
    column appended so the softmax denominator falls out of the PV
    matmul's last output column
  - sequences with context >= 256 store K and V in fp8-E3M4 (their
    softmax averages over many slots, so the ~2% fp8 round-off washes
    out; the PE accepts mixed fp8/bf16 matmul operands); short
    sequences — whose output is nearly a copy of one V row — stay in
    bf16. q stays bf16 with the 1/sqrt(D) scale folded in.

Device (identical program on all 8 cores; chunk offsets baked from the
block tables / context lens, which are shared across heads):
  - PE clock-gate (HAM) warm-up: ~30 back-to-back dummy matmuls at
    kernel start (overlapping the first piece DMAs) cover >= 2 full
    4096-cycle HAM windows so the PE clock promotes 1.2 -> 2.4 GHz
    before real work lands (it re-promotes on its own later; sustained
    full-rate matmul is power-throttled to ~40-50% at 2.4 GHz).
  - stream K/V in pieces (piece boundaries at sequence boundaries),
    then per seq b, chunk j:
      scoresT[s, g] = sum_d KT[d, s] * qd[d, (b,g)]     (PE -> PSUM)
    expT = exp(scoresT) -> bf16                         (ACT -> SBUF)
    per chunk: out[g, d|1] += expT[s, g]^T @ V1[s, d|1] (PE, PSUM acc)
    out[g, :D] * (1 / out[g, D])                        (DVE)
No max-subtraction in the softmax: q,k ~ N(0,1) so scores ~ N(0,1) and
exp() stays in a tiny fp32/bf16 range. Total round-off ~1.1e-2 vs the
fp32 reference (gate is 2e-2).
"""

import sys

for _p in ("/opt/trn_rl_repo", "/opt/pypackages"):
    if _p not in sys.path:
        sys.path.insert(0, _p)

import numpy as np

import concourse.bass as bass
import concourse.mybir as mybir
import concourse.tile as tile
from concourse.bass_utils import run_bass_kernel_spmd

B = 64
H = 32
KVH = 8
D = 128
BS = 128
NBPS = 16
NUM_BLOCKS = B * NBPS
SCALE = 1.0 / np.float32(np.sqrt(D))
N_CORES = 8
G = H // KVH  # query heads per kv head (= per core)

PIECE_CHUNKS = 32   # chunks per streaming DMA piece
K8POOL_BUFS = 5
KBPOOL_BUFS = 2
V8POOL_BUFS = 5
VBPOOL_BUFS = 2
SPSUM_BUFS = 4
OPSUM_BUFS = 3
EXP_BUFS = 6
WARMUP_MM = 16      # dummy matmuls to unthrottle the PE clock gate
PV_LAG = 2
FP8_CTX_CUT = 256   # sequences at least this long stream K/V in fp8


def _split_waits_bir_json(bir: bytes) -> bytes:
    """This container's walrus build accepts only ONE sync-wait per
    instruction (setupSyncWait raises "Too many sync wait commands"),
    while Tile freely attaches several. Rewrite the BIR: hoist all but
    the last wait of each instruction onto single-wait NOPs inserted
    immediately before it on the same engine (same-engine program order
    makes this semantically identical)."""
    import orjson

    j = orjson.loads(bir)
    changed = False
    for f in j.get("functions", []):
        for bb in f.get("blocks", []):
            insts = bb.get("instructions", [])
            out = []
            for inst in insts:
                waits = (inst.get("sync_info") or {}).get("on_wait") or []
                if len(waits) > 1:
                    changed = True
                    for kk, w in enumerate(waits[:-1]):
                        out.append({
                            "engine": inst["engine"],
                            "ins": [],
                            "name": f"{inst['name']}-ws{kk}",
                            "opcode": "NoOp",
                            "outs": [],
                            "sync_info": {"on_update": [], "on_wait": [w]},
                        })
                    inst["sync_info"]["on_wait"] = [waits[-1]]
                out.append(inst)
            bb["instructions"] = out
    return orjson.dumps(j) if changed else bir


_orig_compile_bir_kernel = None


def _install_compile_patch():
    global _orig_compile_bir_kernel
    import concourse.bass2jax as bass2jax
    import concourse.bass_utils as bass_utils

    if _orig_compile_bir_kernel is not None:
        return
    _orig_compile_bir_kernel = bass_utils.compile_bir_kernel

    def patched(bir_json, tmpdir, neff_name="file.neff"):
        if isinstance(bir_json, str):
            bir_json = bir_json.encode()
        return _orig_compile_bir_kernel(
            _split_waits_bir_json(bir_json), tmpdir, neff_name=neff_name
        )

    bass_utils.compile_bir_kernel = patched
    bass2jax.compile_bir_kernel = patched


def _make_plan(context_lens):
    """Chunk bookkeeping shared by host layout and device program.

    Per sequence: n chunks, fp8 class, and the chunk-prefix within its
    class's K/V streams. Pieces are runs of consecutive seqs.
    """
    n_blocks = [-(-int(c) // BS) for c in context_lens]
    is8 = [int(c) >= FP8_CTX_CUT for c in context_lens]
    prefix = [0]
    for n in n_blocks:
        prefix.append(prefix[-1] + n)
    total_chunks = prefix[-1]
    cpre = []  # class-local chunk prefix per seq
    c8 = cb = 0
    for b in range(B):
        if is8[b]:
            cpre.append(c8)
            c8 += n_blocks[b]
        else:
            cpre.append(cb)
            cb += n_blocks[b]
    tot8, totb = c8, cb

    caps = [24, 32]
    pieces = []  # (first_seq, last_seq_exclusive, chunk_start, n_chunks)
    b0 = 0
    while b0 < B:
        if len(pieces) < len(caps):
            cap = caps[len(pieces)]  # head: big pieces hide DGE ramp-up
        else:
            rem = total_chunks - prefix[b0]
            # tail ramp: small final pieces so the last data lands while
            # the PV/normalize pipeline is still draining earlier seqs
            cap = PIECE_CHUNKS if rem > 56 else (16 if rem > 24 else 8)
        b1 = b0
        nch = 0
        while b1 < B and (nch + n_blocks[b1] <= cap or b1 == b0):
            nch += n_blocks[b1]
            b1 += 1
        assert b1 > b0
        pieces.append((b0, b1, prefix[b0], nch))
        b0 = b1
    return n_blocks, is8, prefix, cpre, tot8, totb, pieces


def _build_program(plan, ctx_lens):
    """One SPMD program for all cores (offsets are shared across cores)."""
    n_blocks, is8, prefix, cpre, tot8, totb, pieces = plan
    nc = bass.Bass("TRN2", target_bir_lowering=False, debug=False)
    k8 = nc.dram_tensor("k8", [D, max(tot8, 1) * BS], mybir.dt.float8e3,
                        kind="ExternalInput")
    kb = nc.dram_tensor("kb", [D, max(totb, 1) * BS], mybir.dt.bfloat16,
                        kind="ExternalInput")
    v8 = nc.dram_tensor("v8", [BS, max(tot8, 1) * (D + 1)],
                        mybir.dt.float8e3, kind="ExternalInput")
    vb = nc.dram_tensor("vb", [BS, max(totb, 1) * (D + 1)],
                        mybir.dt.bfloat16, kind="ExternalInput")
    qd = nc.dram_tensor("qd", [D, B * G], mybir.dt.bfloat16,
                        kind="ExternalInput")
    out = nc.dram_tensor("out", [G, B * D], mybir.dt.float32,
                         kind="ExternalOutput")
    k8_ap, kb_ap, v8_ap, vb_ap = k8.ap(), kb.ap(), v8.ap(), vb.ap()
    qd_ap, out_ap = qd.ap(), out.ap()

    with tile.TileContext(nc) as tc:
        with (
            tc.tile_pool(name="singles", bufs=1) as singles,
            tc.tile_pool(name="k8pool", bufs=K8POOL_BUFS) as k8pool,
            tc.tile_pool(name="kbpool", bufs=KBPOOL_BUFS) as kbpool,
            tc.tile_pool(name="v8pool", bufs=V8POOL_BUFS) as v8pool,
            tc.tile_pool(name="vbpool", bufs=VBPOOL_BUFS) as vbpool,
            tc.tile_pool(name="epool", bufs=EXP_BUFS) as epool,
            tc.tile_pool(name="rpool", bufs=4) as rpool,
            tc.tile_pool(name="spsum", bufs=SPSUM_BUFS, space="PSUM") as spsum,
            tc.tile_pool(name="opsum", bufs=OPSUM_BUFS, space="PSUM") as opsum,
            tc.tile_pool(name="wpsum", bufs=1, space="PSUM") as wpsum,
        ):
            qd_t = singles.tile([D, B * G], mybir.dt.bfloat16)
            # qd rides SWDGE so its descriptor generation doesn't delay
            # the first K piece's on the sync queue
            nc.gpsimd.dma_start(out=qd_t, in_=qd_ap[:, :])
            out_all = singles.tile([G, B * D], mybir.dt.float32)

            # HAM warm-up: the PE clock gate promotes 1.2 -> 2.4 GHz only
            # after a fully-busy 4096-cycle (3.4us) activity window. Run
            # ~6.4us of back-to-back dummy matmuls (covers >= 2 windows
            # at any phase) while the first DMA pieces are in flight.
            # The source is a locally memset scratch tile, not qd, so the
            # burst starts right after the preamble instead of waiting
            # for the qd DMA (and constant data keeps MAC toggling low).
            wsrc = singles.tile([D, B * G], mybir.dt.bfloat16, tag="wsrc")
            nc.vector.memset(wsrc, 1.0)
            warm = wpsum.tile([128, B * G], mybir.dt.float32, tag="warm")
            for _ in range(WARMUP_MM):
                nc.tensor.matmul(
                    warm,
                    lhsT=wsrc[:, 0:128],
                    rhs=wsrc[:, 0:B * G],
                    start=True, stop=True,
                )

            # Software-pipelined emission: PV for seq b is emitted PV_LAG
            # sequences after its QK, so by the time the PE queue reaches
            # it, the exp chain has finished and PV doesn't head-of-
            # line-block ready QK work behind it.
            pending = []

            def emit_pv(ent):
                b, n, r, lv, et, ot, v_tile = ent
                for j in range(n):
                    m = BS if j < n - 1 else r
                    co = (lv + j) * (D + 1)
                    nc.tensor.matmul(
                        ot,
                        lhsT=et[0:m, 4 * j:4 * j + 4],
                        rhs=v_tile[0:m, co:co + D + 1],
                        start=(j == 0), stop=(j == n - 1),
                    )
                rc = rpool.tile([G, 1], mybir.dt.float32, tag="rc")
                nc.vector.reciprocal(out=rc, in_=ot[:, D:D + 1])
                nc.vector.tensor_scalar_mul(
                    out=out_all[:, D * b:D * (b + 1)],
                    in0=ot[:, 0:D],
                    scalar1=rc,
                )
                # stream results out in quarters (eighths near the end,
                # so the final DMA waits on fewer sequences)
                flushes = {16: 0, 32: 16, 48: 32, 56: 48, 64: 56}
                if (b + 1) in flushes:
                    q0 = flushes[b + 1] * D
                    nc.sync.dma_start(
                        out=out_ap[:, q0:(b + 1) * D],
                        in_=out_all[:, q0:(b + 1) * D],
                    )

            for (b0, b1, c0, nch) in pieces:
                # per-class K/V slabs for this piece (each class's chunks
                # are contiguous in its streams because pieces are runs
                # of consecutive seqs)
                n8 = sum(n_blocks[b] for b in range(b0, b1) if is8[b])
                nb = nch - n8
                c8_0 = next((cpre[b] for b in range(b0, b1) if is8[b]), 0)
                cb_0 = next((cpre[b] for b in range(b0, b1) if not is8[b]), 0)
                k8_t = kb_t = v8_t = vb_t = None
                if n8:
                    k8_t = k8pool.tile([D, PIECE_CHUNKS * BS],
                                       mybir.dt.float8e3, tag="k8piece")
                    nc.sync.dma_start(
                        out=k8_t[:, 0:n8 * BS],
                        in_=k8_ap[:, c8_0 * BS:(c8_0 + n8) * BS],
                    )
                    v8_t = v8pool.tile([BS, PIECE_CHUNKS * (D + 1)],
                                       mybir.dt.float8e3, tag="v8piece")
                    # V triggers ride the idle GPSIMD queue (SWDGE):
                    # descriptor generation runs in parallel with the
                    # K stream's HWDGE on the sync queue, and exp ops
                    # on the ACT queue never stall behind a trigger
                    nc.gpsimd.dma_start(
                        out=v8_t[:, 0:n8 * (D + 1)],
                        in_=v8_ap[:, c8_0 * (D + 1):(c8_0 + n8) * (D + 1)],
                    )
                if nb:
                    kb_t = kbpool.tile([D, PIECE_CHUNKS * BS],
                                       mybir.dt.bfloat16, tag="kbpiece")
                    nc.sync.dma_start(
                        out=kb_t[:, 0:nb * BS],
                        in_=kb_ap[:, cb_0 * BS:(cb_0 + nb) * BS],
                    )
                    vb_t = vbpool.tile([BS, PIECE_CHUNKS * (D + 1)],
                                       mybir.dt.bfloat16, tag="vbpiece")
                    nc.gpsimd.dma_start(
                        out=vb_t[:, 0:nb * (D + 1)],
                        in_=vb_ap[:, cb_0 * (D + 1):(cb_0 + nb) * (D + 1)],
                    )

                for b in range(b0, b1):
                    n = n_blocks[b]
                    r = int(ctx_lens[b]) - BS * (n - 1)
                    if is8[b]:
                        lv, k_tile, v_tile = cpre[b] - c8_0, k8_t, v8_t
                    else:
                        lv, k_tile, v_tile = cpre[b] - cb_0, kb_t, vb_t
                    st = spsum.tile([BS, 4 * n], mybir.dt.float32, tag="st")
                    et = epool.tile([BS, 4 * n], mybir.dt.bfloat16, tag="et")
                    ot = opsum.tile([G, D + 1], mybir.dt.float32, tag="ot")

                    for j in range(n):
                        m = BS if j < n - 1 else r
                        co = (lv + j) * BS
                        nc.tensor.matmul(
                            st[0:m, 4 * j:4 * j + 4],
                            lhsT=k_tile[:, co:co + m],
                            rhs=qd_t[:, 4 * b:4 * b + 4],
                            start=True, stop=True,
                        )

                    if n > 1:
                        nc.scalar.activation(
                            out=et[:, 0:4 * (n - 1)],
                            in_=st[:, 0:4 * (n - 1)],
                            func=mybir.ActivationFunctionType.Exp,
                        )
                    nc.scalar.activation(
                        out=et[0:r, 4 * (n - 1):4 * n],
                        in_=st[0:r, 4 * (n - 1):4 * n],
                        func=mybir.ActivationFunctionType.Exp,
                    )

                    pending.append((b, n, r, lv, et, ot, v_tile))
                    if len(pending) > PV_LAG:
                        emit_pv(pending.pop(0))

            for ent in pending:
                emit_pv(ent)

    return nc


def kernel(q, k, v, k_cache, v_cache, slot_mapping, block_tables,
           context_lens, _trace=False):
    q = np.asarray(q, dtype=np.float32)
    k = np.asarray(k, dtype=np.float32)
    v = np.asarray(v, dtype=np.float32)
    k_cache = np.asarray(k_cache, dtype=np.float32)
    v_cache = np.asarray(v_cache, dtype=np.float32)
    slot_mapping = np.asarray(slot_mapping)
    block_tables = np.asarray(block_tables)
    context_lens = np.asarray(context_lens)

    blk_of = slot_mapping // BS
    slt_of = slot_mapping % BS

    plan = _make_plan(context_lens)
    n_blocks, is8, prefix, cpre, tot8, totb, pieces = plan
    blk_8 = np.concatenate(
        [block_tables[b, :n_blocks[b]] for b in range(B) if is8[b]]
        or [np.zeros(1, np.int32)]
    ).astype(np.int64)
    blk_b = np.concatenate(
        [block_tables[b, :n_blocks[b]] for b in range(B) if not is8[b]]
        or [np.zeros(1, np.int32)]
    ).astype(np.int64)

    # [kvh, block, d, slot] / [kvh, block, slot, d+1] with token scatter
    kt_all = np.empty((KVH, NUM_BLOCKS, D, BS), dtype=np.float32)
    kt_all[:] = k_cache.transpose(2, 0, 3, 1)
    v1_all = np.empty((KVH, NUM_BLOCKS, BS, D + 1), dtype=np.float32)
    v1_all[:, :, :, :D] = v_cache.transpose(2, 0, 1, 3)
    v1_all[:, :, :, D] = 1.0
    for b in range(B):
        kt_all[:, blk_of[b], :, slt_of[b]] = k[b]
        v1_all[:, blk_of[b], slt_of[b], :D] = v[b]

    qs = (q * SCALE).astype(np.float32)  # [B, H, D]

    import ml_dtypes
    bf16 = ml_dtypes.bfloat16
    f8e3 = ml_dtypes.float8_e3m4

    _install_compile_patch()
    nc = _build_program(plan, context_lens)

    in_maps = []
    for i in range(N_CORES):
        k8_i = kt_all[i, blk_8].transpose(1, 0, 2).reshape(D, -1)
        kb_i = kt_all[i, blk_b].transpose(1, 0, 2).reshape(D, -1)
        v8_i = v1_all[i, blk_8].transpose(1, 0, 2).reshape(BS, -1)
        vb_i = v1_all[i, blk_b].transpose(1, 0, 2).reshape(BS, -1)
        qd_i = qs[:, G * i:G * (i + 1), :].transpose(2, 0, 1).reshape(D, B * G)
        in_maps.append({
            "k8": np.ascontiguousarray(k8_i.astype(f8e3)),
            "kb": np.ascontiguousarray(kb_i.astype(bf16)),
            "v8": np.ascontiguousarray(v8_i.astype(f8e3)),
            "vb": np.ascontiguousarray(vb_i.astype(bf16)),
            "qd": np.ascontiguousarray(qd_i.astype(bf16)),
        })

    res = run_bass_kernel_spmd(
        nc, in_maps, core_ids=list(range(N_CORES)), trace=_trace,
    )

    out = np.empty((B, H, D), dtype=np.float32)
    for i in range(N_CORES):
        o = res.results[i]["out"].reshape(G, B, D)
        out[:, G * i:G * (i + 1), :] = o.transpose(1, 0, 2)

    if _trace:
        kernel._last_result = res
    return out


# revision 23
# speedup vs baseline: 1.0343x; 1.0343x over previous
"""Paged-attention decode (GQA) on 8 Trainium2 NeuronCores.

Sharding: tensor-parallel along the kv-head axis. Core i gets kv head i
and its 4 query heads (H=32, KVH=8 -> G=4), plus all 64 sequences.

Host-side prep (per core) — a per-shard block re-allocator:
  - scatter the new k/v token into the cache shard (store_kvcache)
  - defragment: order each sequence's allocated blocks contiguously,
    dropping blocks past ceil(context_len/128) (never attended)
  - K laid out [d, seq-chunk-major slots] so K^T streams into SBUF
    with d on partitions (the QK^T matmul contracts over d)
  - V laid out [slot-in-chunk, seq-chunk-major (d+1)] with a ones
    column appended so the softmax denominator falls out of the PV
    matmul's last output column
  - sequences with context >= 256 store K and V in fp8-E3M4 (their
    softmax averages over many slots, so the ~2% fp8 round-off washes
    out; the PE accepts mixed fp8/bf16 matmul operands); short
    sequences — whose output is nearly a copy of one V row — stay in
    bf16. q stays bf16 with the 1/sqrt(D) scale folded in.

Device (identical program on all 8 cores; chunk offsets baked from the
block tables / context lens, which are shared across heads):
  - PE clock-gate (HAM) warm-up: ~30 back-to-back dummy matmuls at
    kernel start (overlapping the first piece DMAs) cover >= 2 full
    4096-cycle HAM windows so the PE clock promotes 1.2 -> 2.4 GHz
    before real work lands (it re-promotes on its own later; sustained
    full-rate matmul is power-throttled to ~40-50% at 2.4 GHz).
  - stream K/V in pieces (piece boundaries at sequence boundaries),
    then per seq b, chunk j:
      scoresT[s, g] = sum_d KT[d, s] * qd[d, (b,g)]     (PE -> PSUM)
    expT = exp(scoresT) -> bf16                         (ACT -> SBUF)
    per chunk: out[g, d|1] += expT[s, g]^T @ V1[s, d|1] (PE, PSUM acc)
    out[g, :D] * (1 / out[g, D])                        (DVE)
No max-subtraction in the softmax: q,k ~ N(0,1) so scores ~ N(0,1) and
exp() stays in a tiny fp32/bf16 range. Total round-off ~1.1e-2 vs the
fp32 reference (gate is 2e-2).
"""

import sys

for _p in ("/opt/trn_rl_repo", "/opt/pypackages"):
    if _p not in sys.path:
        sys.path.insert(0, _p)

import numpy as np

import concourse.bass as bass
import concourse.mybir as mybir
import concourse.tile as tile
from concourse.bass_utils import run_bass_kernel_spmd

B = 64
H = 32
KVH = 8
D = 128
BS = 128
NBPS = 16
NUM_BLOCKS = B * NBPS
SCALE = 1.0 / np.float32(np.sqrt(D))
N_CORES = 8
G = H // KVH  # query heads per kv head (= per core)

PIECE_CHUNKS = 32   # chunks per streaming DMA piece
K8POOL_BUFS = 5
KBPOOL_BUFS = 2
V8POOL_BUFS = 5
VBPOOL_BUFS = 2
SPSUM_BUFS = 4
OPSUM_BUFS = 3
EXP_BUFS = 6
WARMUP_MM = 30      # dummy matmuls to unthrottle the PE clock gate
PV_LAG = 2
FP8_CTX_CUT = 256   # sequences at least this long stream K/V in fp8


def _split_waits_bir_json(bir: bytes) -> bytes:
    """This container's walrus build accepts only ONE sync-wait per
    instruction (setupSyncWait raises "Too many sync wait commands"),
    while Tile freely attaches several. Rewrite the BIR: hoist all but
    the last wait of each instruction onto single-wait NOPs inserted
    immediately before it on the same engine (same-engine program order
    makes this semantically identical)."""
    import orjson

    j = orjson.loads(bir)
    changed = False
    for f in j.get("functions", []):
        for bb in f.get("blocks", []):
            insts = bb.get("instructions", [])
            out = []
            for inst in insts:
                waits = (inst.get("sync_info") or {}).get("on_wait") or []
                if len(waits) > 1:
                    changed = True
                    for kk, w in enumerate(waits[:-1]):
                        out.append({
                            "engine": inst["engine"],
                            "ins": [],
                            "name": f"{inst['name']}-ws{kk}",
                            "opcode": "NoOp",
                            "outs": [],
                            "sync_info": {"on_update": [], "on_wait": [w]},
                        })
                    inst["sync_info"]["on_wait"] = [waits[-1]]
                out.append(inst)
            bb["instructions"] = out
    return orjson.dumps(j) if changed else bir


_orig_compile_bir_kernel = None


def _install_compile_patch():
    global _orig_compile_bir_kernel
    import concourse.bass2jax as bass2jax
    import concourse.bass_utils as bass_utils

    if _orig_compile_bir_kernel is not None:
        return
    _orig_compile_bir_kernel = bass_utils.compile_bir_kernel

    def patched(bir_json, tmpdir, neff_name="file.neff"):
        if isinstance(bir_json, str):
            bir_json = bir_json.encode()
        return _orig_compile_bir_kernel(
            _split_waits_bir_json(bir_json), tmpdir, neff_name=neff_name
        )

    bass_utils.compile_bir_kernel = patched
    bass2jax.compile_bir_kernel = patched


def _make_plan(context_lens):
    """Chunk bookkeeping shared by host layout and device program.

    Per sequence: n chunks, fp8 class, and the chunk-prefix within its
    class's K/V streams. Pieces are runs of consecutive seqs.
    """
    n_blocks = [-(-int(c) // BS) for c in context_lens]
    is8 = [int(c) >= FP8_CTX_CUT for c in context_lens]
    prefix = [0]
    for n in n_blocks:
        prefix.append(prefix[-1] + n)
    total_chunks = prefix[-1]
    cpre = []  # class-local chunk prefix per seq
    c8 = cb = 0
    for b in range(B):
        if is8[b]:
            cpre.append(c8)
            c8 += n_blocks[b]
        else:
            cpre.append(cb)
            cb += n_blocks[b]
    tot8, totb = c8, cb

    caps = [24, 32]
    pieces = []  # (first_seq, last_seq_exclusive, chunk_start, n_chunks)
    b0 = 0
    while b0 < B:
        if len(pieces) < len(caps):
            cap = caps[len(pieces)]  # head: big pieces hide DGE ramp-up
        else:
            rem = total_chunks - prefix[b0]
            # tail ramp: small final pieces so the last data lands while
            # the PV/normalize pipeline is still draining earlier seqs
            cap = PIECE_CHUNKS if rem > 56 else (16 if rem > 24 else 8)
        b1 = b0
        nch = 0
        while b1 < B and (nch + n_blocks[b1] <= cap or b1 == b0):
            nch += n_blocks[b1]
            b1 += 1
        assert b1 > b0
        pieces.append((b0, b1, prefix[b0], nch))
        b0 = b1
    return n_blocks, is8, prefix, cpre, tot8, totb, pieces


def _build_program(plan, ctx_lens):
    """One SPMD program for all cores (offsets are shared across cores)."""
    n_blocks, is8, prefix, cpre, tot8, totb, pieces = plan
    nc = bass.Bass("TRN2", target_bir_lowering=False, debug=False)
    k8 = nc.dram_tensor("k8", [D, max(tot8, 1) * BS], mybir.dt.float8e3,
                        kind="ExternalInput")
    kb = nc.dram_tensor("kb", [D, max(totb, 1) * BS], mybir.dt.bfloat16,
                        kind="ExternalInput")
    v8 = nc.dram_tensor("v8", [BS, max(tot8, 1) * (D + 1)],
                        mybir.dt.float8e3, kind="ExternalInput")
    vb = nc.dram_tensor("vb", [BS, max(totb, 1) * (D + 1)],
                        mybir.dt.bfloat16, kind="ExternalInput")
    qd = nc.dram_tensor("qd", [D, B * G], mybir.dt.bfloat16,
                        kind="ExternalInput")
    out = nc.dram_tensor("out", [G, B * D], mybir.dt.float32,
                         kind="ExternalOutput")
    k8_ap, kb_ap, v8_ap, vb_ap = k8.ap(), kb.ap(), v8.ap(), vb.ap()
    qd_ap, out_ap = qd.ap(), out.ap()

    with tile.TileContext(nc) as tc:
        with (
            tc.tile_pool(name="singles", bufs=1) as singles,
            tc.tile_pool(name="k8pool", bufs=K8POOL_BUFS) as k8pool,
            tc.tile_pool(name="kbpool", bufs=KBPOOL_BUFS) as kbpool,
            tc.tile_pool(name="v8pool", bufs=V8POOL_BUFS) as v8pool,
            tc.tile_pool(name="vbpool", bufs=VBPOOL_BUFS) as vbpool,
            tc.tile_pool(name="epool", bufs=EXP_BUFS) as epool,
            tc.tile_pool(name="rpool", bufs=4) as rpool,
            tc.tile_pool(name="spsum", bufs=SPSUM_BUFS, space="PSUM") as spsum,
            tc.tile_pool(name="opsum", bufs=OPSUM_BUFS, space="PSUM") as opsum,
            tc.tile_pool(name="wpsum", bufs=1, space="PSUM") as wpsum,
        ):
            qd_t = singles.tile([D, B * G], mybir.dt.bfloat16)
            # qd rides SWDGE so its descriptor generation doesn't delay
            # the first K piece's on the sync queue
            nc.gpsimd.dma_start(out=qd_t, in_=qd_ap[:, :])
            out_all = singles.tile([G, B * D], mybir.dt.float32)

            # HAM warm-up: the PE clock gate promotes 1.2 -> 2.4 GHz only
            # after a fully-busy 4096-cycle (3.4us) activity window. Run
            # ~6.4us of back-to-back dummy matmuls (covers >= 2 windows
            # at any phase) while the first DMA pieces are in flight.
            # The source is a locally memset scratch tile, not qd, so the
            # burst starts right after the preamble instead of waiting
            # for the qd DMA (and constant data keeps MAC toggling low).
            wsrc = singles.tile([D, B * G], mybir.dt.bfloat16, tag="wsrc")
            nc.vector.memset(wsrc, 1.0)
            warm = wpsum.tile([128, B * G], mybir.dt.float32, tag="warm")
            for _ in range(WARMUP_MM):
                nc.tensor.matmul(
                    warm,
                    lhsT=wsrc[:, 0:128],
                    rhs=wsrc[:, 0:B * G],
                    start=True, stop=True,
                )

            # Software-pipelined emission: PV for seq b is emitted PV_LAG
            # sequences after its QK, so by the time the PE queue reaches
            # it, the exp chain has finished and PV doesn't head-of-
            # line-block ready QK work behind it.
            pending = []

            def emit_pv(ent):
                b, n, r, lv, et, ot, v_tile = ent
                for j in range(n):
                    m = BS if j < n - 1 else r
                    co = (lv + j) * (D + 1)
                    nc.tensor.matmul(
                        ot,
                        lhsT=et[0:m, 4 * j:4 * j + 4],
                        rhs=v_tile[0:m, co:co + D + 1],
                        start=(j == 0), stop=(j == n - 1),
                    )
                rc = rpool.tile([G, 1], mybir.dt.float32, tag="rc")
                nc.vector.reciprocal(out=rc, in_=ot[:, D:D + 1])
                nc.vector.tensor_scalar_mul(
                    out=out_all[:, D * b:D * (b + 1)],
                    in0=ot[:, 0:D],
                    scalar1=rc,
                )
                # stream results out in quarters (eighths near the end,
                # so the final DMA waits on fewer sequences)
                flushes = {16: 0, 32: 16, 48: 32, 56: 48, 64: 56}
                if (b + 1) in flushes:
                    q0 = flushes[b + 1] * D
                    nc.sync.dma_start(
                        out=out_ap[:, q0:(b + 1) * D],
                        in_=out_all[:, q0:(b + 1) * D],
                    )

            for (b0, b1, c0, nch) in pieces:
                # per-class K/V slabs for this piece (each class's chunks
                # are contiguous in its streams because pieces are runs
                # of consecutive seqs)
                n8 = sum(n_blocks[b] for b in range(b0, b1) if is8[b])
                nb = nch - n8
                c8_0 = next((cpre[b] for b in range(b0, b1) if is8[b]), 0)
                cb_0 = next((cpre[b] for b in range(b0, b1) if not is8[b]), 0)
                k8_t = kb_t = v8_t = vb_t = None
                if n8:
                    k8_t = k8pool.tile([D, PIECE_CHUNKS * BS],
                                       mybir.dt.float8e3, tag="k8piece")
                    nc.sync.dma_start(
                        out=k8_t[:, 0:n8 * BS],
                        in_=k8_ap[:, c8_0 * BS:(c8_0 + n8) * BS],
                    )
                    v8_t = v8pool.tile([BS, PIECE_CHUNKS * (D + 1)],
                                       mybir.dt.float8e3, tag="v8piece")
                    # V triggers ride the idle GPSIMD queue (SWDGE):
                    # descriptor generation runs in parallel with the
                    # K stream's HWDGE on the sync queue, and exp ops
                    # on the ACT queue never stall behind a trigger
                    nc.gpsimd.dma_start(
                        out=v8_t[:, 0:n8 * (D + 1)],
                        in_=v8_ap[:, c8_0 * (D + 1):(c8_0 + n8) * (D + 1)],
                    )
                if nb:
                    kb_t = kbpool.tile([D, PIECE_CHUNKS * BS],
                                       mybir.dt.bfloat16, tag="kbpiece")
                    nc.sync.dma_start(
                        out=kb_t[:, 0:nb * BS],
                        in_=kb_ap[:, cb_0 * BS:(cb_0 + nb) * BS],
                    )
                    vb_t = vbpool.tile([BS, PIECE_CHUNKS * (D + 1)],
                                       mybir.dt.bfloat16, tag="vbpiece")
                    nc.gpsimd.dma_start(
                        out=vb_t[:, 0:nb * (D + 1)],
                        in_=vb_ap[:, cb_0 * (D + 1):(cb_0 + nb) * (D + 1)],
                    )

                for b in range(b0, b1):
                    n = n_blocks[b]
                    r = int(ctx_lens[b]) - BS * (n - 1)
                    if is8[b]:
                        lv, k_tile, v_tile = cpre[b] - c8_0, k8_t, v8_t
                    else:
                        lv, k_tile, v_tile = cpre[b] - cb_0, kb_t, vb_t
                    st = spsum.tile([BS, 4 * n], mybir.dt.float32, tag="st")
                    et = epool.tile([BS, 4 * n], mybir.dt.bfloat16, tag="et")
                    ot = opsum.tile([G, D + 1], mybir.dt.float32, tag="ot")

                    for j in range(n):
                        m = BS if j < n - 1 else r
                        co = (lv + j) * BS
                        nc.tensor.matmul(
                            st[0:m, 4 * j:4 * j + 4],
                            lhsT=k_tile[:, co:co + m],
                            rhs=qd_t[:, 4 * b:4 * b + 4],
                            start=True, stop=True,
                        )

                    if n > 1:
                        nc.scalar.activation(
                            out=et[:, 0:4 * (n - 1)],
                            in_=st[:, 0:4 * (n - 1)],
                            func=mybir.ActivationFunctionType.Exp,
                        )
                    nc.scalar.activation(
                        out=et[0:r, 4 * (n - 1):4 * n],
                        in_=st[0:r, 4 * (n - 1):4 * n],
                        func=mybir.ActivationFunctionType.Exp,
                    )

                    pending.append((b, n, r, lv, et, ot, v_tile))
                    if len(pending) > PV_LAG:
                        emit_pv(pending.pop(0))

            for ent in pending:
                emit_pv(ent)

    return nc


def kernel(q, k, v, k_cache, v_cache, slot_mapping, block_tables,
           context_lens, _trace=False):
    q = np.asarray(q, dtype=np.float32)
    k = np.asarray(k, dtype=np.float32)
    v = np.asarray(v, dtype=np.float32)
    k_cache = np.asarray(k_cache, dtype=np.float32)
    v_cache = np.asarray(v_cache, dtype=np.float32)
    slot_mapping = np.asarray(slot_mapping)
    block_tables = np.asarray(block_tables)
    context_lens = np.asarray(context_lens)

    blk_of = slot_mapping // BS
    slt_of = slot_mapping % BS

    plan = _make_plan(context_lens)
    n_blocks, is8, prefix, cpre, tot8, totb, pieces = plan
    blk_8 = np.concatenate(
        [block_tables[b, :n_blocks[b]] for b in range(B) if is8[b]]
        or [np.zeros(1, np.int32)]
    ).astype(np.int64)
    blk_b = np.concatenate(
        [block_tables[b, :n_blocks[b]] for b in range(B) if not is8[b]]
        or [np.zeros(1, np.int32)]
    ).astype(np.int64)

    # [kvh, block, d, slot] / [kvh, block, slot, d+1] with token scatter
    kt_all = np.empty((KVH, NUM_BLOCKS, D, BS), dtype=np.float32)
    kt_all[:] = k_cache.transpose(2, 0, 3, 1)
    v1_all = np.empty((KVH, NUM_BLOCKS, BS, D + 1), dtype=np.float32)
    v1_all[:, :, :, :D] = v_cache.transpose(2, 0, 1, 3)
    v1_all[:, :, :, D] = 1.0
    for b in range(B):
        kt_all[:, blk_of[b], :, slt_of[b]] = k[b]
        v1_all[:, blk_of[b], slt_of[b], :D] = v[b]

    qs = (q * SCALE).astype(np.float32)  # [B, H, D]

    import ml_dtypes
    bf16 = ml_dtypes.bfloat16
    f8e3 = ml_dtypes.float8_e3m4

    _install_compile_patch()
    nc = _build_program(plan, context_lens)

    in_maps = []
    for i in range(N_CORES):
        k8_i = kt_all[i, blk_8].transpose(1, 0, 2).reshape(D, -1)
        kb_i = kt_all[i, blk_b].transpose(1, 0, 2).reshape(D, -1)
        v8_i = v1_all[i, blk_8].transpose(1, 0, 2).reshape(BS, -1)
        vb_i = v1_all[i, blk_b].transpose(1, 0, 2).reshape(BS, -1)
        qd_i = qs[:, G * i:G * (i + 1), :].transpose(2, 0, 1).reshape(D, B * G)
        in_maps.append({
            "k8": np.ascontiguousarray(k8_i.astype(f8e3)),
            "kb": np.ascontiguousarray(kb_i.astype(bf16)),
            "v8": np.ascontiguousarray(v8_i.astype(f8e3)),
            "vb": np.ascontiguousarray(vb_i.astype(bf16)),
            "qd": np.ascontiguousarray(qd_i.astype(bf16)),
        })

    res = run_bass_kernel_spmd(
        nc, in_maps, core_ids=list(range(N_CORES)), trace=_trace,
    )

    out = np.empty((B, H, D), dtype=np.float32)
    for i in range(N_CORES):
        o = res.results[i]["out"].reshape(G, B, D)
        out[:, G * i:G * (i + 1), :] = o.transpose(1, 0, 2)

    if _trace:
        kernel._last_result = res
    return out


# revision 24
# speedup vs baseline: 1.0476x; 1.0128x over previous
"""Paged-attention decode (GQA) on 8 Trainium2 NeuronCores.

Sharding: tensor-parallel along the kv-head axis. Core i gets kv head i
and its 4 query heads (H=32, KVH=8 -> G=4), plus all 64 sequences.

Host-side prep (per core) — a per-shard block re-allocator:
  - scatter the new k/v token into the cache shard (store_kvcache)
  - defragment: order each sequence's allocated blocks contiguously,
    dropping blocks past ceil(context_len/128) (never attended)
  - K laid out [d, seq-chunk-major slots] so K^T streams into SBUF
    with d on partitions (the QK^T matmul contracts over d)
  - V laid out [slot-in-chunk, seq-chunk-major (d+1)] with a ones
    column appended so the softmax denominator falls out of the PV
    matmul's last output column
  - sequences with context >= 256 store K and V in fp8-E3M4 (their
    softmax averages over many slots, so the ~2% fp8 round-off washes
    out; the PE accepts mixed fp8/bf16 matmul operands); short
    sequences — whose output is nearly a copy of one V row — stay in
    bf16. q stays bf16 with the 1/sqrt(D) scale folded in.

Device (identical program on all 8 cores; chunk offsets baked from the
block tables / context lens, which are shared across heads):
  - PE clock-gate (HAM) warm-up: ~30 back-to-back dummy matmuls at
    kernel start (overlapping the first piece DMAs) cover >= 2 full
    4096-cycle HAM windows so the PE clock promotes 1.2 -> 2.4 GHz
    before real work lands (it re-promotes on its own later; sustained
    full-rate matmul is power-throttled to ~40-50% at 2.4 GHz).
  - stream K/V in pieces (piece boundaries at sequence boundaries),
    then per seq b, chunk j:
      scoresT[s, g] = sum_d KT[d, s] * qd[d, (b,g)]     (PE -> PSUM)
    expT = exp(scoresT) -> bf16                         (ACT -> SBUF)
    per chunk: out[g, d|1] += expT[s, g]^T @ V1[s, d|1] (PE, PSUM acc)
    out[g, :D] * (1 / out[g, D])                        (DVE)
No max-subtraction in the softmax: q,k ~ N(0,1) so scores ~ N(0,1) and
exp() stays in a tiny fp32/bf16 range. Total round-off ~1.1e-2 vs the
fp32 reference (gate is 2e-2).
"""

import sys

for _p in ("/opt/trn_rl_repo", "/opt/pypackages"):
    if _p not in sys.path:
        sys.path.insert(0, _p)

import numpy as np

import concourse.bass as bass
import concourse.mybir as mybir
import concourse.tile as tile
from concourse.bass_utils import run_bass_kernel_spmd

B = 64
H = 32
KVH = 8
D = 128
BS = 128
NBPS = 16
NUM_BLOCKS = B * NBPS
SCALE = 1.0 / np.float32(np.sqrt(D))
N_CORES = 8
G = H // KVH  # query heads per kv head (= per core)

PIECE_CHUNKS = 32   # chunks per streaming DMA piece
K8POOL_BUFS = 5
KBPOOL_BUFS = 2
V8POOL_BUFS = 5
VBPOOL_BUFS = 2
SPSUM_BUFS = 5
OPSUM_BUFS = 3
EXP_BUFS = 6
WARMUP_MM = 30      # dummy matmuls to unthrottle the PE clock gate
PV_LAG = 2
FP8_CTX_CUT = 256   # sequences at least this long stream K/V in fp8


def _split_waits_bir_json(bir: bytes) -> bytes:
    """This container's walrus build accepts only ONE sync-wait per
    instruction (setupSyncWait raises "Too many sync wait commands"),
    while Tile freely attaches several. Rewrite the BIR: hoist all but
    the last wait of each instruction onto single-wait NOPs inserted
    immediately before it on the same engine (same-engine program order
    makes this semantically identical)."""
    import orjson

    j = orjson.loads(bir)
    changed = False
    for f in j.get("functions", []):
        for bb in f.get("blocks", []):
            insts = bb.get("instructions", [])
            out = []
            for inst in insts:
                waits = (inst.get("sync_info") or {}).get("on_wait") or []
                if len(waits) > 1:
                    changed = True
                    for kk, w in enumerate(waits[:-1]):
                        out.append({
                            "engine": inst["engine"],
                            "ins": [],
                            "name": f"{inst['name']}-ws{kk}",
                            "opcode": "NoOp",
                            "outs": [],
                            "sync_info": {"on_update": [], "on_wait": [w]},
                        })
                    inst["sync_info"]["on_wait"] = [waits[-1]]
                out.append(inst)
            bb["instructions"] = out
    return orjson.dumps(j) if changed else bir


_orig_compile_bir_kernel = None


def _install_compile_patch():
    global _orig_compile_bir_kernel
    import concourse.bass2jax as bass2jax
    import concourse.bass_utils as bass_utils

    if _orig_compile_bir_kernel is not None:
        return
    _orig_compile_bir_kernel = bass_utils.compile_bir_kernel

    def patched(bir_json, tmpdir, neff_name="file.neff"):
        if isinstance(bir_json, str):
            bir_json = bir_json.encode()
        return _orig_compile_bir_kernel(
            _split_waits_bir_json(bir_json), tmpdir, neff_name=neff_name
        )

    bass_utils.compile_bir_kernel = patched
    bass2jax.compile_bir_kernel = patched


def _make_plan(context_lens):
    """Chunk bookkeeping shared by host layout and device program.

    Per sequence: n chunks, fp8 class, and the chunk-prefix within its
    class's K/V streams. Pieces are runs of consecutive seqs.
    """
    n_blocks = [-(-int(c) // BS) for c in context_lens]
    is8 = [int(c) >= FP8_CTX_CUT for c in context_lens]
    prefix = [0]
    for n in n_blocks:
        prefix.append(prefix[-1] + n)
    total_chunks = prefix[-1]
    cpre = []  # class-local chunk prefix per seq
    c8 = cb = 0
    for b in range(B):
        if is8[b]:
            cpre.append(c8)
            c8 += n_blocks[b]
        else:
            cpre.append(cb)
            cb += n_blocks[b]
    tot8, totb = c8, cb

    caps = [24, 32]
    pieces = []  # (first_seq, last_seq_exclusive, chunk_start, n_chunks)
    b0 = 0
    while b0 < B:
        if len(pieces) < len(caps):
            cap = caps[len(pieces)]  # head: big pieces hide DGE ramp-up
        else:
            rem = total_chunks - prefix[b0]
            # tail ramp: small final pieces so the last data lands while
            # the PV/normalize pipeline is still draining earlier seqs
            cap = PIECE_CHUNKS if rem > 56 else (16 if rem > 24 else 8)
        b1 = b0
        nch = 0
        while b1 < B and (nch + n_blocks[b1] <= cap or b1 == b0):
            nch += n_blocks[b1]
            b1 += 1
        assert b1 > b0
        pieces.append((b0, b1, prefix[b0], nch))
        b0 = b1
    return n_blocks, is8, prefix, cpre, tot8, totb, pieces


def _build_program(plan, ctx_lens):
    """One SPMD program for all cores (offsets are shared across cores)."""
    n_blocks, is8, prefix, cpre, tot8, totb, pieces = plan
    nc = bass.Bass("TRN2", target_bir_lowering=False, debug=False)
    k8 = nc.dram_tensor("k8", [D, max(tot8, 1) * BS], mybir.dt.float8e3,
                        kind="ExternalInput")
    kb = nc.dram_tensor("kb", [D, max(totb, 1) * BS], mybir.dt.bfloat16,
                        kind="ExternalInput")
    v8 = nc.dram_tensor("v8", [BS, max(tot8, 1) * (D + 1)],
                        mybir.dt.float8e3, kind="ExternalInput")
    vb = nc.dram_tensor("vb", [BS, max(totb, 1) * (D + 1)],
                        mybir.dt.bfloat16, kind="ExternalInput")
    qd = nc.dram_tensor("qd", [D, B * G], mybir.dt.bfloat16,
                        kind="ExternalInput")
    out = nc.dram_tensor("out", [G, B * D], mybir.dt.float32,
                         kind="ExternalOutput")
    k8_ap, kb_ap, v8_ap, vb_ap = k8.ap(), kb.ap(), v8.ap(), vb.ap()
    qd_ap, out_ap = qd.ap(), out.ap()

    with tile.TileContext(nc) as tc:
        with (
            tc.tile_pool(name="singles", bufs=1) as singles,
            tc.tile_pool(name="k8pool", bufs=K8POOL_BUFS) as k8pool,
            tc.tile_pool(name="kbpool", bufs=KBPOOL_BUFS) as kbpool,
            tc.tile_pool(name="v8pool", bufs=V8POOL_BUFS) as v8pool,
            tc.tile_pool(name="vbpool", bufs=VBPOOL_BUFS) as vbpool,
            tc.tile_pool(name="epool", bufs=EXP_BUFS) as epool,
            tc.tile_pool(name="rpool", bufs=4) as rpool,
            tc.tile_pool(name="spsum", bufs=SPSUM_BUFS, space="PSUM") as spsum,
            tc.tile_pool(name="opsum", bufs=OPSUM_BUFS, space="PSUM") as opsum,
        ):
            qd_t = singles.tile([D, B * G], mybir.dt.bfloat16)
            # qd rides SWDGE so its descriptor generation doesn't delay
            # the first K piece's on the sync queue
            nc.gpsimd.dma_start(out=qd_t, in_=qd_ap[:, :])
            out_all = singles.tile([G, B * D], mybir.dt.float32)

            # No HAM warm-up burst: the kernel is PE-saturated once data
            # lands, so the clock gate promotes naturally ~3-7us into the
            # dense scores stream; cold REAL work beats warm dummy work.

            # Software-pipelined emission: PV for seq b is emitted PV_LAG
            # sequences after its QK, so by the time the PE queue reaches
            # it, the exp chain has finished and PV doesn't head-of-
            # line-block ready QK work behind it.
            pending = []

            def emit_pv(ent):
                b, n, r, lv, et, ot, v_tile = ent
                for j in range(n):
                    m = BS if j < n - 1 else r
                    co = (lv + j) * (D + 1)
                    nc.tensor.matmul(
                        ot,
                        lhsT=et[0:m, 4 * j:4 * j + 4],
                        rhs=v_tile[0:m, co:co + D + 1],
                        start=(j == 0), stop=(j == n - 1),
                    )
                rc = rpool.tile([G, 1], mybir.dt.float32, tag="rc")
                nc.vector.reciprocal(out=rc, in_=ot[:, D:D + 1])
                nc.vector.tensor_scalar_mul(
                    out=out_all[:, D * b:D * (b + 1)],
                    in0=ot[:, 0:D],
                    scalar1=rc,
                )
                # stream results out in quarters (eighths near the end,
                # so the final DMA waits on fewer sequences)
                flushes = {16: 0, 32: 16, 48: 32, 56: 48, 64: 56}
                if (b + 1) in flushes:
                    q0 = flushes[b + 1] * D
                    nc.sync.dma_start(
                        out=out_ap[:, q0:(b + 1) * D],
                        in_=out_all[:, q0:(b + 1) * D],
                    )

            for (b0, b1, c0, nch) in pieces:
                # per-class K/V slabs for this piece (each class's chunks
                # are contiguous in its streams because pieces are runs
                # of consecutive seqs)
                n8 = sum(n_blocks[b] for b in range(b0, b1) if is8[b])
                nb = nch - n8
                c8_0 = next((cpre[b] for b in range(b0, b1) if is8[b]), 0)
                cb_0 = next((cpre[b] for b in range(b0, b1) if not is8[b]), 0)
                k8_t = kb_t = v8_t = vb_t = None
                keng = nc.gpsimd if c0 == 0 else nc.sync
                if n8:
                    k8_t = k8pool.tile([D, PIECE_CHUNKS * BS],
                                       mybir.dt.float8e3, tag="k8piece")
                    # first piece via SWDGE: the gpsimd queue reaches its
                    # triggers ~4us before sync clears the preamble barrier
                    keng.dma_start(
                        out=k8_t[:, 0:n8 * BS],
                        in_=k8_ap[:, c8_0 * BS:(c8_0 + n8) * BS],
                    )
                    v8_t = v8pool.tile([BS, PIECE_CHUNKS * (D + 1)],
                                       mybir.dt.float8e3, tag="v8piece")
                    # V triggers ride the idle GPSIMD queue (SWDGE):
                    # descriptor generation runs in parallel with the
                    # K stream's HWDGE on the sync queue, and exp ops
                    # on the ACT queue never stall behind a trigger
                    nc.gpsimd.dma_start(
                        out=v8_t[:, 0:n8 * (D + 1)],
                        in_=v8_ap[:, c8_0 * (D + 1):(c8_0 + n8) * (D + 1)],
                    )
                if nb:
                    kb_t = kbpool.tile([D, PIECE_CHUNKS * BS],
                                       mybir.dt.bfloat16, tag="kbpiece")
                    keng.dma_start(
                        out=kb_t[:, 0:nb * BS],
                        in_=kb_ap[:, cb_0 * BS:(cb_0 + nb) * BS],
                    )
                    vb_t = vbpool.tile([BS, PIECE_CHUNKS * (D + 1)],
                                       mybir.dt.bfloat16, tag="vbpiece")
                    nc.gpsimd.dma_start(
                        out=vb_t[:, 0:nb * (D + 1)],
                        in_=vb_ap[:, cb_0 * (D + 1):(cb_0 + nb) * (D + 1)],
                    )

                for b in range(b0, b1):
                    n = n_blocks[b]
                    r = int(ctx_lens[b]) - BS * (n - 1)
                    if is8[b]:
                        lv, k_tile, v_tile = cpre[b] - c8_0, k8_t, v8_t
                    else:
                        lv, k_tile, v_tile = cpre[b] - cb_0, kb_t, vb_t
                    st = spsum.tile([BS, 4 * n], mybir.dt.float32, tag="st")
                    et = epool.tile([BS, 4 * n], mybir.dt.bfloat16, tag="et")
                    ot = opsum.tile([G, D + 1], mybir.dt.float32, tag="ot")

                    for j in range(n):
                        m = BS if j < n - 1 else r
                        co = (lv + j) * BS
                        nc.tensor.matmul(
                            st[0:m, 4 * j:4 * j + 4],
                            lhsT=k_tile[:, co:co + m],
                            rhs=qd_t[:, 4 * b:4 * b + 4],
                            start=True, stop=True,
                        )

                    if n > 1:
                        nc.scalar.activation(
                            out=et[:, 0:4 * (n - 1)],
                            in_=st[:, 0:4 * (n - 1)],
                            func=mybir.ActivationFunctionType.Exp,
                        )
                    nc.scalar.activation(
                        out=et[0:r, 4 * (n - 1):4 * n],
                        in_=st[0:r, 4 * (n - 1):4 * n],
                        func=mybir.ActivationFunctionType.Exp,
                    )

                    pending.append((b, n, r, lv, et, ot, v_tile))
                    if len(pending) > PV_LAG:
                        emit_pv(pending.pop(0))

            for ent in pending:
                emit_pv(ent)

    return nc


def kernel(q, k, v, k_cache, v_cache, slot_mapping, block_tables,
           context_lens, _trace=False):
    q = np.asarray(q, dtype=np.float32)
    k = np.asarray(k, dtype=np.float32)
    v = np.asarray(v, dtype=np.float32)
    k_cache = np.asarray(k_cache, dtype=np.float32)
    v_cache = np.asarray(v_cache, dtype=np.float32)
    slot_mapping = np.asarray(slot_mapping)
    block_tables = np.asarray(block_tables)
    context_lens = np.asarray(context_lens)

    blk_of = slot_mapping // BS
    slt_of = slot_mapping % BS

    plan = _make_plan(context_lens)
    n_blocks, is8, prefix, cpre, tot8, totb, pieces = plan
    blk_8 = np.concatenate(
        [block_tables[b, :n_blocks[b]] for b in range(B) if is8[b]]
        or [np.zeros(1, np.int32)]
    ).astype(np.int64)
    blk_b = np.concatenate(
        [block_tables[b, :n_blocks[b]] for b in range(B) if not is8[b]]
        or [np.zeros(1, np.int32)]
    ).astype(np.int64)

    # [kvh, block, d, slot] / [kvh, block, slot, d+1] with token scatter
    kt_all = np.empty((KVH, NUM_BLOCKS, D, BS), dtype=np.float32)
    kt_all[:] = k_cache.transpose(2, 0, 3, 1)
    v1_all = np.empty((KVH, NUM_BLOCKS, BS, D + 1), dtype=np.float32)
    v1_all[:, :, :, :D] = v_cache.transpose(2, 0, 1, 3)
    v1_all[:, :, :, D] = 1.0
    for b in range(B):
        kt_all[:, blk_of[b], :, slt_of[b]] = k[b]
        v1_all[:, blk_of[b], slt_of[b], :D] = v[b]

    qs = (q * SCALE).astype(np.float32)  # [B, H, D]

    import ml_dtypes
    bf16 = ml_dtypes.bfloat16
    f8e3 = ml_dtypes.float8_e3m4

    _install_compile_patch()
    nc = _build_program(plan, context_lens)

    in_maps = []
    for i in range(N_CORES):
        k8_i = kt_all[i, blk_8].transpose(1, 0, 2).reshape(D, -1)
        kb_i = kt_all[i, blk_b].transpose(1, 0, 2).reshape(D, -1)
        v8_i = v1_all[i, blk_8].transpose(1, 0, 2).reshape(BS, -1)
        vb_i = v1_all[i, blk_b].transpose(1, 0, 2).reshape(BS, -1)
        qd_i = qs[:, G * i:G * (i + 1), :].transpose(2, 0, 1).reshape(D, B * G)
        in_maps.append({
            "k8": np.ascontiguousarray(k8_i.astype(f8e3)),
            "kb": np.ascontiguousarray(kb_i.astype(bf16)),
            "v8": np.ascontiguousarray(v8_i.astype(f8e3)),
            "vb": np.ascontiguousarray(vb_i.astype(bf16)),
            "qd": np.ascontiguousarray(qd_i.astype(bf16)),
        })

    res = run_bass_kernel_spmd(
        nc, in_maps, core_ids=list(range(N_CORES)), trace=_trace,
    )

    out = np.empty((B, H, D), dtype=np.float32)
    for i in range(N_CORES):
        o = res.results[i]["out"].reshape(G, B, D)
        out[:, G * i:G * (i + 1), :] = o.transpose(1, 0, 2)

    if _trace:
        kernel._last_result = res
    return out
